# revision 22
# baseline (speedup 1.0000x reference)
"""Trainium2 kernel for nn_LinearAutoDecoder (cluster-routed per-row 3x95 matvec).

out[i] = W[3*c_i : 3*c_i+3] @ x_i  with W = [W_pos | W_feat] in R^{384x95}.

Strategy (v2): rows are grouped by cluster and sharded round-robin across the
8 cores (identical static schedule on every core). X streams as fp16 [95, R]
(feature-major, half the HBM bytes of fp32 -- the kernel is memory-bound).
The PE runs one matmul per <=128-sample group with the group's X columns as
the STATIONARY operand and the cluster's 3 weight columns as the MOVING
operand, so outputs land as [samples, 3] across PSUM partitions. That keeps
the PSUM->SBUF copy free-dim tiny (3 cols/group) and the PE moving-dim work
negligible. Output DMAs go out via HWDGE (nc.sync) so they never head-of-line
block the X-load queue (SWDGE/gpsimd). The host scatters [128, 3G] fp16
results back to original row order in fp32.
"""

import os
import sys

for _p in (
    "/root/.axon_site",
    "/root/.axon_site/_ro/trn_rl_repo",
    "/root/.axon_site/_ro/pypackages",
    "/opt/trn_rl_repo",
    "/opt/pypackages",
):
    if os.path.isdir(_p) and _p not in sys.path:
        sys.path.append(_p)

import numpy as np

N_CORES = 8
F = 95            # feature dim (63 pos + 32 latent) = matmul K
NCL = 128         # clusters
CHUNK = 24576     # X columns per DMA chunk
GRP = 128         # samples per matmul group (stationary columns)
MAXG = 128        # groups per PSUM tile (3*128=384 fp32 <= one 2KB bank)
OUT_BATCH = 2     # chunks per output DMA

_prog_cache = {}


def _build_program(chunk_meta, R, OT):
    from contextlib import ExitStack

    import concourse.bacc as bacc
    import concourse.tile as tile
    import concourse.tile_sem_assignment as tsa
    from concourse import mybir

    # Keep the end-of-kernel drain wait fan-in within walrus' per-instruction
    # sync-wait budget: two SWDGE completion lanes instead of eight.
    tsa.NUM_SWDGE_GLOBAL_SEMS = 2

    nc = bacc.Bacc(
        "TRN2", target_bir_lowering=False, debug=False, num_devices=N_CORES
    )
    f16 = mybir.dt.float16
    xt = nc.dram_tensor("xt", [F, R], f16, kind="ExternalInput").ap()
    wt = nc.dram_tensor("wt", [F, 3 * NCL], f16, kind="ExternalInput").ap()
    ix = nc.dram_tensor("ix", [GRP, 8], mybir.dt.int16, kind="ExternalInput").ap()
    ot = nc.dram_tensor("ot", [GRP, OT], f16, kind="ExternalOutput").ap()

    with tile.TileContext(nc, trace_sim=False) as tc, ExitStack() as ctx:
        wpool = ctx.enter_context(tc.tile_pool(name="w", bufs=1))
        xpool = ctx.enter_context(tc.tile_pool(name="x", bufs=3))
        opool = ctx.enter_context(tc.tile_pool(name="o", bufs=4))
        ppool = ctx.enter_context(tc.tile_pool(name="p", bufs=4, space="PSUM"))

        w_sb = wpool.tile([F, 3 * NCL], f16)
        ix_sb = wpool.tile([GRP, 8], mybir.dt.int16, tag="ix")

        n_chunks = len(chunk_meta)
        copy_flip = 0
        # Pair chunks per output DMA (>=512B per-partition contiguous runs).
        batches = []
        cb = 0
        while cb < n_chunks:
            take = min(OUT_BATCH, n_chunks - cb)
            batches.append(chunk_meta[cb : cb + take])
            cb += take
        for bi, batch in enumerate(batches):
            o0 = batch[0]["ocol"]
            o1 = batch[-1]["ocol"] + 3 * sum(
                len(pt) for pt in batch[-1]["ptiles"]
            )
            o_sb = opool.tile([GRP, o1 - o0], f16)
            for meta in batch:
                c0, c1 = meta["c0"], meta["c1"]
                x_sb = xpool.tile([F, c1 - c0], f16)
                # First chunk via HWDGE (sync): ~0.9us faster issue chain
                # than SWDGE, and it frees the Pool queue to pre-generate
                # chunk 1 in parallel. W rides after chunk 0 (matmuls wait
                # on X anyway, so W is never on the critical path).
                if meta is chunk_meta[0]:
                    nc.sync.dma_start(x_sb[:], xt[:, c0:c1])
                    nc.scalar.dma_start(w_sb[:], wt[:])
                    nc.scalar.dma_start(ix_sb[:], ix[:])
                else:
                    nc.gpsimd.dma_start(x_sb[:], xt[:, c0:c1])
                ocol = meta["ocol"]
                for pt in meta["ptiles"]:
                    g = len(pt)
                    ps = ppool.tile([GRP, 3 * g], mybir.dt.float32)
                    for k, (x0, L, c) in enumerate(pt):
                        nc.tensor.matmul(
                            ps[0:L, 3 * k : 3 * k + 3],
                            lhsT=x_sb[:, x0 - c0 : x0 - c0 + L],
                            rhs=w_sb[:, 3 * c : 3 * c + 3],
                            start=True,
                            stop=True,
                        )
                    sl = slice(ocol - o0, ocol - o0 + 3 * g)
                    if copy_flip % 2 == 0:
                        nc.vector.tensor_copy(o_sb[:, sl], ps[:])
                    else:
                        nc.scalar.copy(o_sb[:, sl], ps[:])
                    copy_flip += 1
                    ocol += 3 * g
            if bi == len(batches) - 1:
                # Final flush via SWDGE prepared scatter: descriptors are
                # generated mid-stream (no data dep on the copies — Tile
                # defers that to the trigger), so after the last copy only
                # the trigger + transfer remain on the critical tail.
                dma_sem = nc.alloc_semaphore("final_out_dma")
                nc.gpsimd.dma_scatter_add(
                    ot[:, o0:o1],
                    o_sb[:].unsqueeze(1),
                    ix_sb[:],
                    GRP,
                    GRP,
                    o1 - o0,
                    elem_step=OT,
                    prepare_only=True,
                    sem=dma_sem,
                )
                nc.gpsimd.trigger_dma(count=1)
            else:
                nc.sync.dma_start(ot[:, o0:o1], o_sb[:])
    _fixup_prep_sems(nc)
    nc.compile()
    return nc


def _fixup_prep_sems(nc):
    """Retarget each scatter-prep's descriptor sem (on_update[0]) to the
    dangling Tile DMASW lane sem: Tile's drain waits the prep's DMASW lane
    tick but never attaches the matching inc for DRAM-dst preps, so point
    the descriptor-completion sem at that lane."""
    preps, waits_by, incs_by = [], {}, {}
    for blk in nc.m.functions[0].blocks:
        for ins in blk.instructions:
            if type(ins).__name__ == "InstDMAScatterAddAnt" and ins.gen_mode == 1:
                preps.append(ins)
            si = ins.sync_info
            if not si:
                continue
            for w in si.on_wait:
                if "DMASW" in (w.ant_name or ""):
                    k = (w.id, w.ant_name)
                    waits_by[k] = max(waits_by.get(k, 0), w.wait_value)
            for u in si.on_update:
                if "DMASW" in (u.ant_name or ""):
                    k = (u.id, u.ant_name)
                    incs_by[k] = incs_by.get(k, 0) + 16
    dangling = [k for k in waits_by if waits_by[k] > incs_by.get(k, 0)]
    assert len(dangling) == len(preps), (dangling, len(preps))
    for p, (sid, sname) in zip(preps, dangling):
        u0 = p.sync_info.on_update[0]
        u0.id = sid
        u0.ant_name = sname


def _make_schedule(counts):
    """Identical per-core schedule: cluster c contributes ceil(counts[c]/8)
    columns; groups of <=128 samples split at chunk boundaries."""
    Lc = [(int(counts[c]) + N_CORES - 1) // N_CORES for c in range(NCL)]
    runs = []
    base = 0
    for c in range(NCL):
        if Lc[c]:
            runs.append((base, Lc[c], c))
            base += Lc[c]
    R = (base + 15) // 16 * 16  # pad X cols to 16 (DMA alignment); no groups
    bounds = list(range(0, R, CHUNK)) + [R]
    if bounds[-2] == R:
        bounds.pop()

    groups = []
    for r0, rl, c in runs:
        pos = r0
        end = r0 + rl
        while pos < end:
            import bisect

            bi = bisect.bisect_right(bounds, pos)
            nxt = bounds[bi] if bi < len(bounds) else end
            take = min(GRP, end - pos, nxt - pos)
            groups.append((pos, take, c))
            pos += take

    chunk_meta = []
    ocol = 0
    gi = 0
    for ch in range(len(bounds) - 1):
        c0, c1 = bounds[ch], bounds[ch + 1]
        ptiles = []
        cur = []
        while gi < len(groups) and groups[gi][0] < c1:
            cur.append(groups[gi])
            gi += 1
            if len(cur) == MAXG:
                ptiles.append(cur)
                cur = []
        if cur:
            ptiles.append(cur)
        chunk_meta.append({"c0": c0, "c1": c1, "ptiles": ptiles, "ocol": ocol})
        ocol += 3 * sum(len(pt) for pt in ptiles)
    # The final batch flushes via dma_scatter_add, whose dst row stride
    # (OT elems * 2B) must divide by 256 -> OT % 128 == 0; align its o0
    # to 128 as well so the dst base stays 256B-aligned.
    n_chunks = len(chunk_meta)
    fb0 = 0
    while fb0 + OUT_BATCH < n_chunks:
        fb0 += OUT_BATCH
    o0 = chunk_meta[fb0]["ocol"]
    delta = (-o0) % 128
    for meta in chunk_meta[fb0:]:
        meta["ocol"] += delta
    ocol += delta
    OT = (ocol + 127) // 128 * 128
    return Lc, runs, groups, chunk_meta, R, OT


def kernel(X, cluster_ids, W_pos, W_feat):
    X = np.asarray(X, dtype=np.float32)
    ids = np.asarray(cluster_ids, dtype=np.int32)
    W_pos = np.asarray(W_pos, dtype=np.float32)
    W_feat = np.asarray(W_feat, dtype=np.float32)
    N = X.shape[0]

    W = np.concatenate([W_pos, W_feat], axis=1)  # [384, 95]
    WT = np.ascontiguousarray(W.T.astype(np.float16))  # [95, 384]

    order = np.argsort(ids, kind="stable")
    counts = np.bincount(ids, minlength=NCL)
    offs = np.concatenate([[0], np.cumsum(counts)])

    Lc, runs, groups, chunk_meta, R, OT = _make_schedule(counts)

    # Per-core row lists: cluster c's shard for core m is Ic[m::8], padded to
    # Lc[c] with index N (an all-zero row appended to X).
    rows = np.full((N_CORES, R), N, dtype=np.int64)
    for r0, rl, c in runs:
        Ic = order[offs[c] : offs[c + 1]]
        for m in range(N_CORES):
            sh = Ic[m::N_CORES]
            rows[m, r0 : r0 + len(sh)] = sh

    Xaug = np.zeros((N + 1, F), dtype=np.float16)
    Xaug[:N] = X.astype(np.float16)

    # Identity scatter indices: idx j at [j % 16, j // 16], replicated
    # across the 8 gpsimd Q7 cores (16 partitions each).
    ix16 = np.zeros((16, 8), dtype=np.int16)
    for j in range(GRP):
        ix16[j % 16, j // 16] = j
    IX = np.tile(ix16, (8, 1))

    in_maps = []
    for m in range(N_CORES):
        Xt = np.ascontiguousarray(Xaug[rows[m]].T)  # [95, R] fp16
        in_maps.append({"xt": Xt, "wt": WT, "ix": IX})

    key = (tuple(groups), R, OT)
    if key not in _prog_cache:
        _prog_cache.clear()
        _prog_cache[key] = _build_program(chunk_meta, R, OT)
    nc = _prog_cache[key]

    from concourse.bass_utils import run_bass_kernel_spmd

    res = run_bass_kernel_spmd(nc, in_maps, list(range(N_CORES)))

    # Gather indices shared by all cores: group k owns out cols 3k..3k+3 of
    # its psum tile; flatten (partition, ocol) per sample in schedule order.
    part_idx = np.concatenate([np.arange(L) for (_, L, _) in groups])
    ocols = []
    for meta in chunk_meta:
        oc = meta["ocol"]
        for pt in meta["ptiles"]:
            for k, (_, L, _) in enumerate(pt):
                ocols.append(np.full(L, oc + 3 * k))
            oc += 3 * len(pt)
    col_idx = np.concatenate(ocols)
    xcols = np.concatenate([np.arange(x0, x0 + L) for (x0, L, _) in groups])

    out = np.zeros((N, 3), dtype=np.float32)
    for m in range(N_CORES):
        otm = np.asarray(res.results[m]["ot"])  # [128, OT] fp16
        vals = otm[part_idx[:, None], col_idx[:, None] + np.arange(3)]
        samples = rows[m][xcols]
        valid = samples != N
        out[samples[valid]] = vals[valid].astype(np.float32)
    return out


# revision 23
# speedup vs baseline: 1.0021x; 1.0021x over previous
"""Trainium2 kernel for nn_LinearAutoDecoder (cluster-routed per-row 3x95 matvec).

out[i] = W[3*c_i : 3*c_i+3] @ x_i  with W = [W_pos | W_feat] in R^{384x95}.

Strategy (v2): rows are grouped by cluster and sharded round-robin across the
8 cores (identical static schedule on every core). X streams as fp16 [95, R]
(feature-major, half the HBM bytes of fp32 -- the kernel is memory-bound).
The PE runs one matmul per <=128-sample group with the group's X columns as
the STATIONARY operand and the cluster's 3 weight columns as the MOVING
operand, so outputs land as [samples, 3] across PSUM partitions. That keeps
the PSUM->SBUF copy free-dim tiny (3 cols/group) and the PE moving-dim work
negligible. Output DMAs go out via HWDGE (nc.sync) so they never head-of-line
block the X-load queue (SWDGE/gpsimd). The host scatters [128, 3G] fp16
results back to original row order in fp32.
"""

import os
import sys

for _p in (
    "/root/.axon_site",
    "/root/.axon_site/_ro/trn_rl_repo",
    "/root/.axon_site/_ro/pypackages",
    "/opt/trn_rl_repo",
    "/opt/pypackages",
):
    if os.path.isdir(_p) and _p not in sys.path:
        sys.path.append(_p)

import numpy as np

N_CORES = 8
F = 95            # feature dim (63 pos + 32 latent) = matmul K
NCL = 128         # clusters
CHUNK = 24576     # X columns per DMA chunk
GRP = 128         # samples per matmul group (stationary columns)
MAXG = 128        # groups per PSUM tile (3*128=384 fp32 <= one 2KB bank)
OUT_BATCH = 2     # chunks per output DMA

_prog_cache = {}


def _build_program(chunk_meta, R, OT):
    from contextlib import ExitStack

    import concourse.bacc as bacc
    import concourse.tile as tile
    import concourse.tile_sem_assignment as tsa
    from concourse import mybir

    # Keep the end-of-kernel drain wait fan-in within walrus' per-instruction
    # sync-wait budget: two SWDGE and two HWDGE completion lanes instead of
    # eight of each (fewer preamble clears + drain waits; one lane of either
    # kind over-serializes).
    tsa.NUM_SWDGE_GLOBAL_SEMS = 2
    tsa.NUM_HWDGE_SEMS = 2

    nc = bacc.Bacc(
        "TRN2", target_bir_lowering=False, debug=False, num_devices=N_CORES
    )
    f16 = mybir.dt.float16
    xt = nc.dram_tensor("xt", [F, R], f16, kind="ExternalInput").ap()
    wt = nc.dram_tensor("wt", [F, 3 * NCL], f16, kind="ExternalInput").ap()
    ix = nc.dram_tensor("ix", [GRP, 8], mybir.dt.int16, kind="ExternalInput").ap()
    ot = nc.dram_tensor("ot", [GRP, OT], f16, kind="ExternalOutput").ap()

    with tile.TileContext(nc, trace_sim=False) as tc, ExitStack() as ctx:
        wpool = ctx.enter_context(tc.tile_pool(name="w", bufs=1))
        xpool = ctx.enter_context(tc.tile_pool(name="x", bufs=3))
        opool = ctx.enter_context(tc.tile_pool(name="o", bufs=4))
        ppool = ctx.enter_context(tc.tile_pool(name="p", bufs=4, space="PSUM"))

        w_sb = wpool.tile([F, 3 * NCL], f16)
        ix_sb = wpool.tile([GRP, 8], mybir.dt.int16, tag="ix")

        n_chunks = len(chunk_meta)
        copy_flip = 0
        # Pair chunks per output DMA (>=512B per-partition contiguous runs).
        batches = []
        cb = 0
        while cb < n_chunks:
            take = min(OUT_BATCH, n_chunks - cb)
            batches.append(chunk_meta[cb : cb + take])
            cb += take
        for bi, batch in enumerate(batches):
            o0 = batch[0]["ocol"]
            o1 = batch[-1]["ocol"] + 3 * sum(
                len(pt) for pt in batch[-1]["ptiles"]
            )
            o_sb = opool.tile([GRP, o1 - o0], f16)
            for meta in batch:
                c0, c1 = meta["c0"], meta["c1"]
                x_sb = xpool.tile([F, c1 - c0], f16)
                # First chunk via HWDGE (sync): ~0.9us faster issue chain
                # than SWDGE, and it frees the Pool queue to pre-generate
                # chunk 1 in parallel. W rides after chunk 0 (matmuls wait
                # on X anyway, so W is never on the critical path).
                if meta is chunk_meta[0]:
                    nc.sync.dma_start(x_sb[:], xt[:, c0:c1])
                    nc.scalar.dma_start(w_sb[:], wt[:])
                    nc.scalar.dma_start(ix_sb[:], ix[:])
                else:
                    nc.gpsimd.dma_start(x_sb[:], xt[:, c0:c1])
                ocol = meta["ocol"]
                for pt in meta["ptiles"]:
                    g = len(pt)
                    ps = ppool.tile([GRP, 3 * g], mybir.dt.float32)
                    for k, (x0, L, c) in enumerate(pt):
                        nc.tensor.matmul(
                            ps[0:L, 3 * k : 3 * k + 3],
                            lhsT=x_sb[:, x0 - c0 : x0 - c0 + L],
                            rhs=w_sb[:, 3 * c : 3 * c + 3],
                            start=True,
                            stop=True,
                        )
                    sl = slice(ocol - o0, ocol - o0 + 3 * g)
                    if copy_flip % 2 == 0:
                        nc.vector.tensor_copy(o_sb[:, sl], ps[:])
                    else:
                        nc.scalar.copy(o_sb[:, sl], ps[:])
                    copy_flip += 1
                    ocol += 3 * g
            if bi == len(batches) - 1:
                # Final flush via SWDGE prepared scatter: descriptors are
                # generated mid-stream (no data dep on the copies — Tile
                # defers that to the trigger), so after the last copy only
                # the trigger + transfer remain on the critical tail.
                dma_sem = nc.alloc_semaphore("final_out_dma")
                nc.gpsimd.dma_scatter_add(
                    ot[:, o0:o1],
                    o_sb[:].unsqueeze(1),
                    ix_sb[:],
                    GRP,
                    GRP,
                    o1 - o0,
                    elem_step=OT,
                    prepare_only=True,
                    sem=dma_sem,
                )
                nc.gpsimd.trigger_dma(count=1)
            else:
                nc.sync.dma_start(ot[:, o0:o1], o_sb[:])
    _fixup_prep_sems(nc)
    nc.compile()
    return nc


def _fixup_prep_sems(nc):
    """Retarget each scatter-prep's descriptor sem (on_update[0]) to the
    dangling Tile DMASW lane sem: Tile's drain waits the prep's DMASW lane
    tick but never attaches the matching inc for DRAM-dst preps, so point
    the descriptor-completion sem at that lane."""
    preps, waits_by, incs_by = [], {}, {}
    for blk in nc.m.functions[0].blocks:
        for ins in blk.instructions:
            if type(ins).__name__ == "InstDMAScatterAddAnt" and ins.gen_mode == 1:
                preps.append(ins)
            si = ins.sync_info
            if not si:
                continue
            for w in si.on_wait:
                if "DMASW" in (w.ant_name or ""):
                    k = (w.id, w.ant_name)
                    waits_by[k] = max(waits_by.get(k, 0), w.wait_value)
            for u in si.on_update:
                if "DMASW" in (u.ant_name or ""):
                    k = (u.id, u.ant_name)
                    incs_by[k] = incs_by.get(k, 0) + 16
    dangling = [k for k in waits_by if waits_by[k] > incs_by.get(k, 0)]
    assert len(dangling) == len(preps), (dangling, len(preps))
    for p, (sid, sname) in zip(preps, dangling):
        u0 = p.sync_info.on_update[0]
        u0.id = sid
        u0.ant_name = sname


def _make_schedule(counts):
    """Identical per-core schedule: cluster c contributes ceil(counts[c]/8)
    columns; groups of <=128 samples split at chunk boundaries."""
    Lc = [(int(counts[c]) + N_CORES - 1) // N_CORES for c in range(NCL)]
    runs = []
    base = 0
    for c in range(NCL):
        if Lc[c]:
            runs.append((base, Lc[c], c))
            base += Lc[c]
    R = (base + 15) // 16 * 16  # pad X cols to 16 (DMA alignment); no groups
    bounds = list(range(0, R, CHUNK)) + [R]
    if bounds[-2] == R:
        bounds.pop()

    groups = []
    for r0, rl, c in runs:
        pos = r0
        end = r0 + rl
        while pos < end:
            import bisect

            bi = bisect.bisect_right(bounds, pos)
            nxt = bounds[bi] if bi < len(bounds) else end
            take = min(GRP, end - pos, nxt - pos)
            groups.append((pos, take, c))
            pos += take

    chunk_meta = []
    ocol = 0
    gi = 0
    for ch in range(len(bounds) - 1):
        c0, c1 = bounds[ch], bounds[ch + 1]
        ptiles = []
        cur = []
        while gi < len(groups) and groups[gi][0] < c1:
            cur.append(groups[gi])
            gi += 1
            if len(cur) == MAXG:
                ptiles.append(cur)
                cur = []
        if cur:
            ptiles.append(cur)
        chunk_meta.append({"c0": c0, "c1": c1, "ptiles": ptiles, "ocol": ocol})
        ocol += 3 * sum(len(pt) for pt in ptiles)
    # The final batch flushes via dma_scatter_add, whose dst row stride
    # (OT elems * 2B) must divide by 256 -> OT % 128 == 0; align its o0
    # to 128 as well so the dst base stays 256B-aligned.
    n_chunks = len(chunk_meta)
    fb0 = 0
    while fb0 + OUT_BATCH < n_chunks:
        fb0 += OUT_BATCH
    o0 = chunk_meta[fb0]["ocol"]
    delta = (-o0) % 128
    for meta in chunk_meta[fb0:]:
        meta["ocol"] += delta
    ocol += delta
    OT = (ocol + 127) // 128 * 128
    return Lc, runs, groups, chunk_meta, R, OT


def kernel(X, cluster_ids, W_pos, W_feat):
    X = np.asarray(X, dtype=np.float32)
    ids = np.asarray(cluster_ids, dtype=np.int32)
    W_pos = np.asarray(W_pos, dtype=np.float32)
    W_feat = np.asarray(W_feat, dtype=np.float32)
    N = X.shape[0]

    W = np.concatenate([W_pos, W_feat], axis=1)  # [384, 95]
    WT = np.ascontiguousarray(W.T.astype(np.float16))  # [95, 384]

    order = np.argsort(ids, kind="stable")
    counts = np.bincount(ids, minlength=NCL)
    offs = np.concatenate([[0], np.cumsum(counts)])

    Lc, runs, groups, chunk_meta, R, OT = _make_schedule(counts)

    # Per-core row lists: cluster c's shard for core m is Ic[m::8], padded to
    # Lc[c] with index N (an all-zero row appended to X).
    rows = np.full((N_CORES, R), N, dtype=np.int64)
    for r0, rl, c in runs:
        Ic = order[offs[c] : offs[c + 1]]
        for m in range(N_CORES):
            sh = Ic[m::N_CORES]
            rows[m, r0 : r0 + len(sh)] = sh

    Xaug = np.zeros((N + 1, F), dtype=np.float16)
    Xaug[:N] = X.astype(np.float16)

    # Identity scatter indices: idx j at [j % 16, j // 16], replicated
    # across the 8 gpsimd Q7 cores (16 partitions each).
    ix16 = np.zeros((16, 8), dtype=np.int16)
    for j in range(GRP):
        ix16[j % 16, j // 16] = j
    IX = np.tile(ix16, (8, 1))

    in_maps = []
    for m in range(N_CORES):
        Xt = np.ascontiguousarray(Xaug[rows[m]].T)  # [95, R] fp16
        in_maps.append({"xt": Xt, "wt": WT, "ix": IX})

    key = (tuple(groups), R, OT)
    if key not in _prog_cache:
        _prog_cache.clear()
        _prog_cache[key] = _build_program(chunk_meta, R, OT)
    nc = _prog_cache[key]

    from concourse.bass_utils import run_bass_kernel_spmd

    res = run_bass_kernel_spmd(nc, in_maps, list(range(N_CORES)))

    # Gather indices shared by all cores: group k owns out cols 3k..3k+3 of
    # its psum tile; flatten (partition, ocol) per sample in schedule order.
    part_idx = np.concatenate([np.arange(L) for (_, L, _) in groups])
    ocols = []
    for meta in chunk_meta:
        oc = meta["ocol"]
        for pt in meta["ptiles"]:
            for k, (_, L, _) in enumerate(pt):
                ocols.append(np.full(L, oc + 3 * k))
            oc += 3 * len(pt)
    col_idx = np.concatenate(ocols)
    xcols = np.concatenate([np.arange(x0, x0 + L) for (x0, L, _) in groups])

    out = np.zeros((N, 3), dtype=np.float32)
    for m in range(N_CORES):
        otm = np.asarray(res.results[m]["ot"])  # [128, OT] fp16
        vals = otm[part_idx[:, None], col_idx[:, None] + np.arange(3)]
        samples = rows[m][xcols]
        valid = samples != N
        out[samples[valid]] = vals[valid].astype(np.float32)
    return out


# revision 26
# speedup vs baseline: 1.0032x; 1.0011x over previous
"""Trainium2 kernel for nn_LinearAutoDecoder (cluster-routed per-row 3x95 matvec).

out[i] = W[3*c_i : 3*c_i+3] @ x_i  with W = [W_pos | W_feat] in R^{384x95}.

Strategy (v2): rows are grouped by cluster and sharded round-robin across the
8 cores (identical static schedule on every core). X streams as fp16 [95, R]
(feature-major, half the HBM bytes of fp32 -- the kernel is memory-bound).
The PE runs one matmul per <=128-sample group with the group's X columns as
the STATIONARY operand and the cluster's 3 weight columns as the MOVING
operand, so outputs land as [samples, 3] across PSUM partitions. That keeps
the PSUM->SBUF copy free-dim tiny (3 cols/group) and the PE moving-dim work
negligible. Output DMAs go out via HWDGE (nc.sync) so they never head-of-line
block the X-load queue (SWDGE/gpsimd). The host scatters [128, 3G] fp16
results back to original row order in fp32.
"""

import os
import sys

for _p in (
    "/root/.axon_site",
    "/root/.axon_site/_ro/trn_rl_repo",
    "/root/.axon_site/_ro/pypackages",
    "/opt/trn_rl_repo",
    "/opt/pypackages",
):
    if os.path.isdir(_p) and _p not in sys.path:
        sys.path.append(_p)

import numpy as np

N_CORES = 8
F = 95            # feature dim (63 pos + 32 latent) = matmul K
NCL = 128         # clusters
CHUNK = 24576     # X columns per DMA chunk
GRP = 128         # samples per matmul group (stationary columns)
MAXG = 128        # groups per PSUM tile (3*128=384 fp32 <= one 2KB bank)
OUT_BATCH = 2     # chunks per output DMA

_prog_cache = {}


def _build_program(chunk_meta, R, OT):
    from contextlib import ExitStack

    import concourse.bacc as bacc
    import concourse.tile as tile
    import concourse.tile_sem_assignment as tsa
    from concourse import mybir

    # Keep the end-of-kernel drain wait fan-in within walrus' per-instruction
    # sync-wait budget: two SWDGE and two HWDGE completion lanes instead of
    # eight of each (fewer preamble clears + drain waits; one lane of either
    # kind over-serializes).
    tsa.NUM_SWDGE_GLOBAL_SEMS = 2
    tsa.NUM_HWDGE_SEMS = 2

    nc = bacc.Bacc(
        "TRN2", target_bir_lowering=False, debug=False, num_devices=N_CORES
    )
    f16 = mybir.dt.float16
    xt = nc.dram_tensor("xt", [F, R], f16, kind="ExternalInput").ap()
    wt = nc.dram_tensor("wt", [F, 3 * NCL], f16, kind="ExternalInput").ap()
    ix = nc.dram_tensor("ix", [GRP, 8], mybir.dt.int16, kind="ExternalInput").ap()
    ot = nc.dram_tensor("ot", [GRP, OT], f16, kind="ExternalOutput").ap()

    with tile.TileContext(nc, trace_sim=False) as tc, ExitStack() as ctx:
        wpool = ctx.enter_context(tc.tile_pool(name="w", bufs=1))
        xpool = ctx.enter_context(tc.tile_pool(name="x", bufs=3))
        opool = ctx.enter_context(tc.tile_pool(name="o", bufs=4))
        ppool = ctx.enter_context(tc.tile_pool(name="p", bufs=4, space="PSUM"))

        w_sb = wpool.tile([F, 3 * NCL], f16)
        ix_sb = wpool.tile([GRP, 8], mybir.dt.int16, tag="ix")

        n_chunks = len(chunk_meta)
        copy_flip = 0
        # Batch plan anchored at the end: the final batch is always the
        # last two chunks (the prepared-scatter payload); the rest pair up
        # front-to-back, with any lone leftover being a big chunk whose
        # copies finish early.
        batches = []
        cb = 0
        while cb < max(n_chunks - 2, 0):
            take = min(OUT_BATCH, max(n_chunks - 2, 0) - cb)
            batches.append(chunk_meta[cb : cb + take])
            cb += take
        batches.append(chunk_meta[cb:])
        for bi, batch in enumerate(batches):
            o0 = batch[0]["ocol"]
            o1 = batch[-1]["ocol"] + 3 * sum(
                len(pt) for pt in batch[-1]["ptiles"]
            )
            o_sb = opool.tile([GRP, o1 - o0], f16)
            for meta in batch:
                c0, c1 = meta["c0"], meta["c1"]
                x_sb = xpool.tile([F, c1 - c0], f16)
                # First chunk via HWDGE (sync): ~0.9us faster issue chain
                # than SWDGE, and it frees the Pool queue to pre-generate
                # chunk 1 in parallel. W rides after chunk 0 (matmuls wait
                # on X anyway, so W is never on the critical path).
                if meta is chunk_meta[0]:
                    nc.sync.dma_start(x_sb[:], xt[:, c0:c1])
                    nc.scalar.dma_start(w_sb[:], wt[:])
                    nc.scalar.dma_start(ix_sb[:], ix[:])
                else:
                    nc.gpsimd.dma_start(x_sb[:], xt[:, c0:c1])
                ocol = meta["ocol"]
                for pt in meta["ptiles"]:
                    g = len(pt)
                    ps = ppool.tile([GRP, 3 * g], mybir.dt.float32)
                    for k, (x0, L, c) in enumerate(pt):
                        nc.tensor.matmul(
                            ps[0:L, 3 * k : 3 * k + 3],
                            lhsT=x_sb[:, x0 - c0 : x0 - c0 + L],
                            rhs=w_sb[:, 3 * c : 3 * c + 3],
                            start=True,
                            stop=True,
                        )
                    sl = slice(ocol - o0, ocol - o0 + 3 * g)
                    if copy_flip % 2 == 0:
                        nc.vector.tensor_copy(o_sb[:, sl], ps[:])
                    else:
                        nc.scalar.copy(o_sb[:, sl], ps[:])
                    copy_flip += 1
                    ocol += 3 * g
            if bi == len(batches) - 1:
                # Final flush via SWDGE prepared scatter: descriptors are
                # generated mid-stream (no data dep on the copies — Tile
                # defers that to the trigger), so after the last copy only
                # the trigger + transfer remain on the critical tail.
                dma_sem = nc.alloc_semaphore("final_out_dma")
                nc.gpsimd.dma_scatter_add(
                    ot[:, o0:o1],
                    o_sb[:].unsqueeze(1),
                    ix_sb[:],
                    GRP,
                    GRP,
                    o1 - o0,
                    elem_step=OT,
                    prepare_only=True,
                    sem=dma_sem,
                )
                nc.gpsimd.trigger_dma(count=1)
            else:
                nc.sync.dma_start(ot[:, o0:o1], o_sb[:])
    _fixup_prep_sems(nc)
    nc.compile()
    return nc


def _fixup_prep_sems(nc):
    """Retarget each scatter-prep's descriptor sem (on_update[0]) to the
    dangling Tile DMASW lane sem: Tile's drain waits the prep's DMASW lane
    tick but never attaches the matching inc for DRAM-dst preps, so point
    the descriptor-completion sem at that lane."""
    preps, waits_by, incs_by = [], {}, {}
    for blk in nc.m.functions[0].blocks:
        for ins in blk.instructions:
            if type(ins).__name__ == "InstDMAScatterAddAnt" and ins.gen_mode == 1:
                preps.append(ins)
            si = ins.sync_info
            if not si:
                continue
            for w in si.on_wait:
                if "DMASW" in (w.ant_name or ""):
                    k = (w.id, w.ant_name)
                    waits_by[k] = max(waits_by.get(k, 0), w.wait_value)
            for u in si.on_update:
                if "DMASW" in (u.ant_name or ""):
                    k = (u.id, u.ant_name)
                    incs_by[k] = incs_by.get(k, 0) + 16
    dangling = [k for k in waits_by if waits_by[k] > incs_by.get(k, 0)]
    assert len(dangling) == len(preps), (dangling, len(preps))
    for p, (sid, sname) in zip(preps, dangling):
        u0 = p.sync_info.on_update[0]
        u0.id = sid
        u0.ant_name = sname


def _make_schedule(counts):
    """Identical per-core schedule: cluster c contributes ceil(counts[c]/8)
    columns; groups of <=128 samples split at chunk boundaries."""
    Lc = [(int(counts[c]) + N_CORES - 1) // N_CORES for c in range(NCL)]
    runs = []
    base = 0
    for c in range(NCL):
        if Lc[c]:
            runs.append((base, Lc[c], c))
            base += Lc[c]
    R = (base + 15) // 16 * 16  # pad X cols to 16 (DMA alignment); no groups
    # Four big chunks + two small tail chunks: the last batch (the two
    # tail chunks) is the prepared-scatter payload, so small tail chunks
    # shrink the final transfer on the critical tail; total bytes unchanged.
    # Four big chunks + two smaller tail chunks; the tail pair is the
    # prepared-scatter batch, so the final transfer payload (and the
    # last chunk's mm+copy chain) stays small on the critical tail.
    tail2 = 2048
    tail1 = 16384
    if R > 4 * 16 + tail1 + tail2:
        B = (R - tail1 - tail2) // 4 // 16 * 16
        tail1 = R - tail2 - 4 * B
        bounds = [0, B, 2 * B, 3 * B, 4 * B, 4 * B + tail1, R]
    else:
        bounds = list(range(0, R, CHUNK)) + [R]
        if bounds[-2] == R:
            bounds.pop()

    groups = []
    for r0, rl, c in runs:
        pos = r0
        end = r0 + rl
        while pos < end:
            import bisect

            bi = bisect.bisect_right(bounds, pos)
            nxt = bounds[bi] if bi < len(bounds) else end
            take = min(GRP, end - pos, nxt - pos)
            groups.append((pos, take, c))
            pos += take

    chunk_meta = []
    ocol = 0
    gi = 0
    for ch in range(len(bounds) - 1):
        c0, c1 = bounds[ch], bounds[ch + 1]
        ptiles = []
        cur = []
        while gi < len(groups) and groups[gi][0] < c1:
            cur.append(groups[gi])
            gi += 1
            if len(cur) == MAXG:
                ptiles.append(cur)
                cur = []
        if cur:
            ptiles.append(cur)
        chunk_meta.append({"c0": c0, "c1": c1, "ptiles": ptiles, "ocol": ocol})
        ocol += 3 * sum(len(pt) for pt in ptiles)
    # The final batch flushes via dma_scatter_add, whose dst row stride
    # (OT elems * 2B) must divide by 256 -> OT % 128 == 0; align its o0
    # to 128 as well so the dst base stays 256B-aligned.
    n_chunks = len(chunk_meta)
    fb0 = 0
    while fb0 + OUT_BATCH < n_chunks:
        fb0 += OUT_BATCH
    o0 = chunk_meta[fb0]["ocol"]
    delta = (-o0) % 128
    for meta in chunk_meta[fb0:]:
        meta["ocol"] += delta
    ocol += delta
    OT = (ocol + 127) // 128 * 128
    return Lc, runs, groups, chunk_meta, R, OT


def kernel(X, cluster_ids, W_pos, W_feat):
    X = np.asarray(X, dtype=np.float32)
    ids = np.asarray(cluster_ids, dtype=np.int32)
    W_pos = np.asarray(W_pos, dtype=np.float32)
    W_feat = np.asarray(W_feat, dtype=np.float32)
    N = X.shape[0]

    W = np.concatenate([W_pos, W_feat], axis=1)  # [384, 95]
    WT = np.ascontiguousarray(W.T.astype(np.float16))  # [95, 384]

    order = np.argsort(ids, kind="stable")
    counts = np.bincount(ids, minlength=NCL)
    offs = np.concatenate([[0], np.cumsum(counts)])

    Lc, runs, groups, chunk_meta, R, OT = _make_schedule(counts)

    # Per-core row lists: cluster c's shard for core m is Ic[m::8], padded to
    # Lc[c] with index N (an all-zero row appended to X).
    rows = np.full((N_CORES, R), N, dtype=np.int64)
    for r0, rl, c in runs:
        Ic = order[offs[c] : offs[c + 1]]
        for m in range(N_CORES):
            sh = Ic[m::N_CORES]
            rows[m, r0 : r0 + len(sh)] = sh

    Xaug = np.zeros((N + 1, F), dtype=np.float16)
    Xaug[:N] = X.astype(np.float16)

    # Identity scatter indices: idx j at [j % 16, j // 16], replicated
    # across the 8 gpsimd Q7 cores (16 partitions each).
    ix16 = np.zeros((16, 8), dtype=np.int16)
    for j in range(GRP):
        ix16[j % 16, j // 16] = j
    IX = np.tile(ix16, (8, 1))

    in_maps = []
    for m in range(N_CORES):
        Xt = np.ascontiguousarray(Xaug[rows[m]].T)  # [95, R] fp16
        in_maps.append({"xt": Xt, "wt": WT, "ix": IX})

    key = (tuple(groups), R, OT)
    if key not in _prog_cache:
        _prog_cache.clear()
        _prog_cache[key] = _build_program(chunk_meta, R, OT)
    nc = _prog_cache[key]

    from concourse.bass_utils import run_bass_kernel_spmd

    res = run_bass_kernel_spmd(nc, in_maps, list(range(N_CORES)))

    # Gather indices shared by all cores: group k owns out cols 3k..3k+3 of
    # its psum tile; flatten (partition, ocol) per sample in schedule order.
    part_idx = np.concatenate([np.arange(L) for (_, L, _) in groups])
    ocols = []
    for meta in chunk_meta:
        oc = meta["ocol"]
        for pt in meta["ptiles"]:
            for k, (_, L, _) in enumerate(pt):
                ocols.append(np.full(L, oc + 3 * k))
            oc += 3 * len(pt)
    col_idx = np.concatenate(ocols)
    xcols = np.concatenate([np.arange(x0, x0 + L) for (x0, L, _) in groups])

    out = np.zeros((N, 3), dtype=np.float32)
    for m in range(N_CORES):
        otm = np.asarray(res.results[m]["ot"])  # [128, OT] fp16
        vals = otm[part_idx[:, None], col_idx[:, None] + np.arange(3)]
        samples = rows[m][xcols]
        valid = samples != N
        out[samples[valid]] = vals[valid].astype(np.float32)
    return out
